# revision 24
# baseline (speedup 1.0000x reference)
"""Fused attention + FC + residual + LayerNorm for Trainium2, 8 NeuronCores.

Problem: B=8, L=2048, d_k=d_v=64, d_model=1024, fp32 I/O.
Sharding: pure data parallel - batch element b -> core b. No collectives.

Key algebraic trick: LayerNorm is scale-invariant, so the softmax
normalization is never applied. With u = PV_unnorm @ W^T (unnormalized
attention output through the FC) and D the per-row softmax denominator:

    LN(u/D + b + r)  ==  LN(u + D*(b + r))

so instead of dividing by D (which needs a reciprocal plus a costly
cross-partition broadcast), the kernel scales the residual by D - a cheap
per-partition tensor_scalar - and lets LN absorb the factor. Only the LN
epsilon needs rescaling (eps' = D^2 * eps), also per-partition.

This target's PE streams at 1.2 GHz (HAM never unthrottles), so matmul cost
is ~0.82 ns/column + ~73 ns/instruction; all K=64 matmuls are row-packed
(two concurrent matmuls in row groups 0-63 / 64-127 via tile_position) at
the cost of duplicating qT/kT/outU/fc_wT across both partition halves.

Software pipeline (engine queues are strict FIFO):

  iter s:  dance(s-1) -> epilogue(s-1) -> attention(s)

  attention(s): per k-tile pair, S^T [128k, 2x512q] via one row-packed
    matmul pair (bf16, f32 PSUM) -> exp on ScalarE (temperature 1/sqrt(64)
    folded into the free affine scale, bf16 out) -> PV matmul accumulates
    [65, 512] f32, row 64 = softmax denominator (ones-column appended to V).
  dance(s): D row -> SBUF f32; out_aug evacuated to bf16 outU (frees the
    PSUM bank immediately) + duplicated across partition halves; D
    transposed to per-partition [128, 4] via 4 tiny K=1 PE matmuls (no
    DRAM roundtrip); eps' = D^2*eps.
  epilogue(s): row-packed FC -> residual scaled by D (DVE tensor_scalar)
    -> added to FC PSUM -> bn_stats/bn_aggr -> rsqrt batched per slice as
    Ln/Exp on ScalarE (single ACT table set for the whole kernel) -> LN
    apply alternating DVE tensor_scalar / ScalarE Identity -> store on the
    GPSIMD SWDGE ring.
"""
import numpy as np

B = 8
L = 2048
D = 64
DM = 1024
NTILES = L // 128       # 16 q/k tiles of 128
NSLICES = L // 512      # 4 q-slices of 512
LN_EPS = 1e-5
SCALE = 0.125           # 1/sqrt(64)

_CACHE = {}
_TABLES_PATCHED = False


def _patch_act_tables():
    """Force every activation we use into one table set so the scheduler
    never needs a mid-kernel ACT_TABLE_LOAD switch (Exp <-> Ln)."""
    global _TABLES_PATCHED
    if _TABLES_PATCHED:
        return
    import concourse.bacc as bacc
    from concourse import mybir

    orig = bacc.get_activation_tables
    keep = "natural_log_exp_and_others"
    shared = {
        mybir.ActivationFunctionType.Exp,
        mybir.ActivationFunctionType.Ln,
        mybir.ActivationFunctionType.Copy,
        mybir.ActivationFunctionType.Identity,
        mybir.ActivationFunctionType.Square,
    }

    def patched(arch):
        tables = orig(arch)
        for name, fns in tables.items():
            if name != keep:
                fns.difference_update(shared)
        return tables

    bacc.get_activation_tables = patched
    _TABLES_PATCHED = True


def _build(affine: bool, with_bias: bool):
    import concourse.bacc as bacc
    import concourse.tile as tile
    from concourse import mybir
    import concourse.bass as bass
    from concourse.masks import make_identity

    _patch_act_tables()
    f32 = mybir.dt.float32
    bf16 = mybir.dt.bfloat16
    nc = bacc.Bacc("TRN2", target_bir_lowering=False, debug=False, num_devices=B)

    q_d = nc.declare_dram_parameter("q", [L, D], f32, isOutput=False)
    k_d = nc.declare_dram_parameter("k", [L, D], f32, isOutput=False)
    v_d = nc.declare_dram_parameter("v", [L, D], f32, isOutput=False)
    res_d = nc.declare_dram_parameter("residual", [L, DM], f32, isOutput=False)
    fcw_d = nc.declare_dram_parameter("fc_w", [DM, D], f32, isOutput=False)
    fcb_d = nc.declare_dram_parameter("fc_b", [DM], f32, isOutput=False)
    gam_d = nc.declare_dram_parameter("ln_gamma", [DM], f32, isOutput=False)
    bet_d = nc.declare_dram_parameter("ln_beta", [DM], f32, isOutput=False)
    out_d = nc.declare_dram_parameter("out", [L, DM], f32, isOutput=True)

    with tile.TileContext(nc) as tc:
        with (
            tc.tile_pool(name="raw", bufs=2) as raw_pool,
            tc.tile_pool(name="persist", bufs=1) as persist,
            tc.tile_pool(name="stage", bufs=2, space="PSUM") as stage_pool,
            tc.tile_pool(name="pv", bufs=2, space="PSUM") as pv_pool,
            tc.tile_pool(name="fc", bufs=1, space="PSUM") as fc_pool,
            tc.tile_pool(name="et", bufs=6) as et_pool,
            tc.tile_pool(name="resid", bufs=6) as res_pool,
            tc.tile_pool(name="x", bufs=10) as x_pool,
            tc.tile_pool(name="rd", bufs=3) as rd_pool,
            tc.tile_pool(name="outs", bufs=4) as out_pool,
            tc.tile_pool(name="norm", bufs=2) as norm_pool,
            tc.tile_pool(name="small", bufs=4) as small_pool,
        ):
            identity = persist.tile([128, 128], f32)
            make_identity(nc, identity)
            one_c = persist.tile([1, 1], f32, tag="onec")
            nc.vector.memset(one_c, 1.0)

            # ---- v load first (SWDGE ring, many small descriptors) ----
            # permuted k-row order matching kT_p: row(tau=2a+b, p) = 16p+2a+b
            vraw = raw_pool.tile([128, NTILES, D], f32, tag="vraw")
            nc.gpsimd.dma_start(
                out=vraw,
                in_=bass.AP(tensor=v_d, offset=0,
                            ap=[[16 * D, 128], [2 * D, 8], [D, 2], [1, D]]),
            )

            # ---- contiguous q/k loads + PE transposes, permuted storage --
            # raw[p, (rr, d)] = src[16p + rr, d]; transpose slice j covers
            # rows (2j, 2j+1) of every p. Storage qT_p[d, j, r, p] holds
            # src-row 16p + 2j + r transposed; all downstream APs use the
            # same (j, r, p) position order, so nothing is ever unpermuted.
            # rows 64:127 duplicate rows 0:63 for row-packed K=64 matmuls.
            qT2 = persist.tile([128, 8, 2, 128], bf16, tag="qT")
            kT2 = persist.tile([128, 8, 2, 128], bf16, tag="kT")
            for ring, (src, dstT) in zip((nc.sync, nc.scalar),
                                         ((q_d, qT2), (k_d, kT2))):
                raw = raw_pool.tile([128, NTILES * D], f32, tag="raw")
                ring.dma_start(
                    out=raw,
                    in_=src.ap().rearrange("(p r) d -> p (r d)", p=128),
                )
                for z in range(2):
                    pt = stage_pool.tile([128, 512], f32, tag="stage")
                    for i in range(4):
                        j = 4 * z + i
                        nc.tensor.transpose(
                            pt[:, i * 128:(i + 1) * 128],
                            raw[:, j * 128:(j + 1) * 128],
                            identity,
                        )
                    ptv = pt.rearrange("p (four c) -> p four c", c=128)
                    nc.vector.tensor_copy(
                        dstT[0:64, 4 * z:4 * z + 4, 0, :], ptv[0:64])
                    nc.vector.tensor_copy(
                        dstT[0:64, 4 * z:4 * z + 4, 1, :], ptv[64:128])
                nc.gpsimd.dma_start(out=dstT[64:128, :, :, :],
                                    in_=dstT[0:64, :, :, :])

            # ---- v with ones column: [128, 16, 65] bf16 ----
            v_sb = persist.tile([128, NTILES, D + 1], bf16, tag="v")
            nc.scalar.copy(v_sb[:, :, 0:D], vraw)
            nc.vector.memset(v_sb[:, :, D:D + 1], 1.0)

            # ---- fc_wT [128, 1024] bf16, duplicated for row-packed FC ----
            fcwT = persist.tile([128, DM], bf16, tag="fcw")
            fraw = raw_pool.tile([128, DM // 128, D], f32, tag="raw")
            nc.sync.dma_start(
                out=fraw, in_=fcw_d.ap().rearrange("(t p) d -> p t d", p=128)
            )
            flo = fcwT[0:64, :].rearrange("d (pair par c) -> d pair par c",
                                          par=2, c=128)
            pt = stage_pool.tile([128, 512], f32, tag="stage")
            for i in range(4):
                nc.tensor.transpose(
                    pt[:, i * 128:(i + 1) * 128],
                    fraw[:, 2 * i: 2 * i + 2, :],
                    identity,
                )
            ptv = pt.rearrange("p (four c) -> p four c", c=128)
            nc.vector.tensor_copy(flo[:, :, 0, :], ptv[0:64])
            nc.vector.tensor_copy(flo[:, :, 1, :], ptv[64:128])
            nc.gpsimd.dma_start(out=fcwT[64:128, :], in_=fcwT[0:64, :])

            if with_bias:
                # residual gets fc_b added per tile (slow path)
                fcb_bc = persist.tile([128, DM], f32, tag="fcb")
                nc.sync.dma_start(
                    out=fcb_bc,
                    in_=bass.AP(tensor=fcb_d, offset=0, ap=[[0, 128], [1, DM]]),
                )
            if affine:
                gam_bc = persist.tile([128, DM], f32, tag="gam")
                bet_bc = persist.tile([128, DM], f32, tag="bet")
                nc.sync.dma_start(
                    out=gam_bc,
                    in_=bass.AP(tensor=gam_d, offset=0, ap=[[0, 128], [1, DM]]),
                )
                nc.sync.dma_start(
                    out=bet_bc,
                    in_=bass.AP(tensor=bet_d, offset=0, ap=[[0, 128], [1, DM]]),
                )

            state = {}

            def attention(s):
                qlo = qT2[0:64, :, :, 32 * s:32 * s + 32]
                qhi = qT2[64:128, :, :, 32 * s:32 * s + 32]
                out_aug = pv_pool.tile([65, 512], f32, tag="pv")
                ngrp = NTILES // 2

                def s_pair(g):
                    # row-packed: k-tile 2g in rows 0:63, 2g+1 in 64:127
                    st = stage_pool.tile([128, 1024], f32, tag="stage")
                    nc.tensor.matmul(st[:, 0:512], kT2[0:64, g, 0, :], qlo,
                                     start=True, stop=True,
                                     tile_position=(0, 0))
                    nc.tensor.matmul(st[:, 512:1024],
                                     kT2[64:128, g, 1, :],
                                     qhi, start=True, stop=True,
                                     tile_position=(64, 0))
                    return st

                def exp_pv(g, st):
                    et = et_pool.tile([128, 1024], bf16, tag="et")
                    nc.scalar.activation(
                        out=et, in_=st,
                        func=mybir.ActivationFunctionType.Exp, scale=SCALE,
                    )
                    nc.tensor.matmul(out_aug, v_sb[:, 2 * g, :], et[:, 0:512],
                                     start=(g == 0), stop=False)
                    nc.tensor.matmul(out_aug, v_sb[:, 2 * g + 1, :],
                                     et[:, 512:1024],
                                     start=False, stop=(g == ngrp - 1))

                # S one group ahead so the PE never waits on exp
                st_prev = s_pair(0)
                for g in range(1, ngrp):
                    st_cur = s_pair(g)
                    exp_pv(g - 1, st_prev)
                    st_prev = st_cur
                exp_pv(ngrp - 1, st_prev)
                return out_aug

            def dance(s, out_aug):
                # f32 denominator row, then evacuate + duplicate (bf16)
                drow = small_pool.tile([1, 512], f32, tag="drow")
                nc.vector.tensor_copy(drow, out_aug[64:65, :])
                outU = norm_pool.tile([128, 512], bf16, tag="outU")
                nc.vector.tensor_copy(outU[0:64, :], out_aug[0:64, :])
                nc.scalar.dma_start(out=outU[64:128, :], in_=outU[0:64, :])
                # D -> per-partition [128, 4] via 4 tiny K=1 PE matmuls
                dps = stage_pool.tile([128, 4], f32, tag="stage")
                for t in range(4):
                    nc.tensor.matmul(dps[:, t:t + 1],
                                     drow[:, t * 128:(t + 1) * 128], one_c,
                                     start=True, stop=True)
                dT = small_pool.tile([128, 4], f32, tag="dT")
                nc.vector.tensor_copy(dT, dps)
                # eps' = D^2 * eps (per-partition epsilon for LN on y=D*x)
                epsT = small_pool.tile([128, 4], f32, tag="epsT")
                nc.vector.tensor_mul(epsT, dT, dT)
                nc.vector.tensor_scalar_mul(out=epsT, in0=epsT,
                                            scalar1=LN_EPS)
                state[s] = {"outU": outU, "dT": dT, "epsT": epsT}

            def epilogue_a(s):
                outU = state[s]["outU"]
                dT = state[s]["dT"]
                mv_all = small_pool.tile([128, 4, 2], f32, tag="mv")
                x_ts = []
                for pi in range(4):
                    t = s * 4 + pi
                    fc_ps = fc_pool.tile([128, DM], f32, tag="fc")
                    nc.tensor.matmul(fc_ps[:, 0:512],
                                     outU[0:64, pi * 128:(pi + 1) * 128],
                                     fcwT[0:64, 0:512],
                                     start=True, stop=True,
                                     tile_position=(0, 0))
                    nc.tensor.matmul(fc_ps[:, 512:1024],
                                     outU[64:128, pi * 128:(pi + 1) * 128],
                                     fcwT[64:128, 512:1024],
                                     start=True, stop=True,
                                     tile_position=(64, 0))
                    res_t = res_pool.tile([128, DM], f32, tag="res")
                    # fc-out partition m=(jl, r, pl) <-> row 512s+4pi+2jl+r+16pl
                    nc.sync.dma_start(
                        out=res_t,
                        in_=bass.AP(tensor=res_d,
                                    offset=(512 * s + 4 * pi) * DM,
                                    ap=[[2 * DM, 2], [DM, 2],
                                        [16 * DM, 32], [1, DM]]),
                    )
                    if with_bias:
                        nc.vector.tensor_add(res_t, res_t, fcb_bc)
                    # y = u + D*(b + r); LN(y) == LN(u/D + b + r)
                    rD = rd_pool.tile([128, DM], f32, tag="rd")
                    nc.vector.tensor_scalar_mul(out=rD, in0=res_t,
                                                scalar1=dT[:, pi:pi + 1])
                    x_t = x_pool.tile([128, DM], f32, tag="x")
                    nc.vector.tensor_add(x_t, fc_ps, rD)
                    x_ts.append(x_t)
                    stats = small_pool.tile([128, 2, 6], f32, tag="stats")
                    nc.vector.bn_stats(out=stats[:, 0, :], in_=x_t[:, 0:512])
                    nc.vector.bn_stats(out=stats[:, 1, :],
                                       in_=x_t[:, 512:1024])
                    nc.vector.bn_aggr(out=mv_all[:, pi, :], in_=stats)
                state[s]["mv"] = mv_all
                state[s]["x_ts"] = x_ts

            def epilogue_b(s):
                epsT = state[s]["epsT"]
                mv_all = state[s]["mv"]
                x_ts = state[s]["x_ts"]
                # batched rsqrt: rstd = exp(-0.5*ln(var + D^2 eps))
                var4 = small_pool.tile([128, 4], f32, tag="var4")
                nc.vector.tensor_add(var4, mv_all[:, :, 1], epsT)
                rstd4 = small_pool.tile([128, 4], f32, tag="rstd")
                nc.scalar.activation(
                    out=rstd4, in_=var4,
                    func=mybir.ActivationFunctionType.Ln,
                )
                nc.scalar.activation(
                    out=rstd4, in_=rstd4,
                    func=mybir.ActivationFunctionType.Exp, scale=-0.5,
                )
                nm4 = small_pool.tile([128, 4], f32, tag="nm")
                nc.vector.tensor_tensor(
                    out=nm4, in0=mv_all[:, :, 0], in1=rstd4,
                    op=mybir.AluOpType.mult,
                )
                nc.vector.tensor_scalar_mul(out=nm4, in0=nm4, scalar1=-1.0)

                for pi in range(4):
                    t = s * 4 + pi
                    out_t = out_pool.tile([128, DM], f32, tag="out")
                    if pi % 2 == 0:
                        nc.vector.tensor_scalar(
                            out=out_t, in0=x_ts[pi],
                            scalar1=mv_all[:, pi, 0:1],
                            scalar2=rstd4[:, pi:pi + 1],
                            op0=mybir.AluOpType.subtract,
                            op1=mybir.AluOpType.mult,
                        )
                    else:
                        nc.scalar.activation(
                            out=out_t, in_=x_ts[pi],
                            func=mybir.ActivationFunctionType.Identity,
                            bias=nm4[:, pi:pi + 1],
                            scale=rstd4[:, pi:pi + 1],
                        )
                    if affine:
                        nc.vector.tensor_mul(out_t, out_t, gam_bc)
                        nc.vector.tensor_add(out_t, out_t, bet_bc)
                    nc.gpsimd.dma_start(
                        out=bass.AP(tensor=out_d,
                                    offset=(512 * s + 4 * pi) * DM,
                                    ap=[[2 * DM, 2], [DM, 2],
                                        [16 * DM, 32], [1, DM]]),
                        in_=out_t,
                    )
                del state[s]

            # pipeline: previous slice's dance + epilogue_a are emitted
            # before the next attention (front of the FIFOs); the LN applies
            # (epilogue_b) go one further iteration later so they never
            # block the following slice's exps on the ScalarE FIFO
            oa = {}
            for s in range(NSLICES + 2):
                if s - 2 >= 0:
                    epilogue_b(s - 2)
                if s - 1 >= 0 and (s - 1) < NSLICES:
                    dance(s - 1, oa.pop(s - 1))
                    epilogue_a(s - 1)
                if s < NSLICES:
                    oa[s] = attention(s)

    nc.finalize()
    return nc


LAST_RESULTS = None


def kernel(q, k, v, residual, fc_w, fc_b, ln_gamma, ln_beta):
    from concourse.bass_utils import run_bass_kernel_spmd

    global LAST_RESULTS
    affine = not (
        np.allclose(ln_gamma, 1.0) and np.allclose(ln_beta, 0.0)
    )
    with_bias = not np.all(np.asarray(fc_b) == 0.0)
    key = ("v11", affine, with_bias)
    if key not in _CACHE:
        _CACHE[key] = _build(affine, with_bias)
    nc = _CACHE[key]

    q = np.ascontiguousarray(q, dtype=np.float32)
    k = np.ascontiguousarray(k, dtype=np.float32)
    v = np.ascontiguousarray(v, dtype=np.float32)
    residual = np.ascontiguousarray(residual, dtype=np.float32)
    fc_w = np.ascontiguousarray(fc_w, dtype=np.float32)
    fc_b = np.ascontiguousarray(fc_b, dtype=np.float32)
    ln_gamma = np.ascontiguousarray(ln_gamma, dtype=np.float32)
    ln_beta = np.ascontiguousarray(ln_beta, dtype=np.float32)

    in_maps = [
        {
            "q": q[b], "k": k[b], "v": v[b], "residual": residual[b],
            "fc_w": fc_w, "fc_b": fc_b,
            "ln_gamma": ln_gamma, "ln_beta": ln_beta,
        }
        for b in range(B)
    ]
    res = run_bass_kernel_spmd(nc, in_maps, core_ids=list(range(B)))
    LAST_RESULTS = res
    return np.stack([res.results[b]["out"] for b in range(B)], axis=0)


# revision 25
# speedup vs baseline: 1.3369x; 1.3369x over previous
"""Fused attention + FC + residual + LayerNorm for Trainium2, 8 NeuronCores.

Problem: B=8, L=2048, d_k=d_v=64, d_model=1024, fp32 I/O.
Sharding: pure data parallel - batch element b -> core b. No collectives.

Key algebraic trick: LayerNorm is scale-invariant, so the softmax
normalization is never applied. With u = PV_unnorm @ W^T (unnormalized
attention output through the FC) and D the per-row softmax denominator:

    LN(u/D + b + r)  ==  LN(u + D*(b + r))

so instead of dividing by D (which needs a reciprocal plus a costly
cross-partition broadcast), the kernel scales the residual by D - a cheap
per-partition tensor_scalar - and lets LN absorb the factor. Only the LN
epsilon needs rescaling (eps' = D^2 * eps), also per-partition.

This target's PE streams at 1.2 GHz (HAM never unthrottles), so matmul cost
is ~0.82 ns/column + ~73 ns/instruction; all K=64 matmuls are row-packed
(two concurrent matmuls in row groups 0-63 / 64-127 via tile_position) at
the cost of duplicating qT/kT/outU/fc_wT across both partition halves.

Software pipeline (engine queues are strict FIFO):

  iter s:  dance(s-1) -> epilogue(s-1) -> attention(s)

  attention(s): per k-tile pair, S^T [128k, 2x512q] via one row-packed
    matmul pair (bf16, f32 PSUM) -> exp on ScalarE (temperature 1/sqrt(64)
    folded into the free affine scale, bf16 out) -> PV matmul accumulates
    [65, 512] f32, row 64 = softmax denominator (ones-column appended to V).
  dance(s): D row -> SBUF f32; out_aug evacuated to bf16 outU (frees the
    PSUM bank immediately) + duplicated across partition halves; D
    transposed to per-partition [128, 4] via 4 tiny K=1 PE matmuls (no
    DRAM roundtrip); eps' = D^2*eps.
  epilogue(s): row-packed FC -> residual scaled by D (DVE tensor_scalar)
    -> added to FC PSUM -> bn_stats/bn_aggr -> rsqrt batched per slice as
    Ln/Exp on ScalarE (single ACT table set for the whole kernel) -> LN
    apply alternating DVE tensor_scalar / ScalarE Identity -> store on the
    GPSIMD SWDGE ring.
"""
import numpy as np

B = 8
L = 2048
D = 64
DM = 1024
NTILES = L // 128       # 16 q/k tiles of 128
NSLICES = L // 512      # 4 q-slices of 512
LN_EPS = 1e-5
SCALE = 0.125           # 1/sqrt(64)

_CACHE = {}
_TABLES_PATCHED = False


def _patch_act_tables():
    """Force every activation we use into one table set so the scheduler
    never needs a mid-kernel ACT_TABLE_LOAD switch (Exp <-> Ln)."""
    global _TABLES_PATCHED
    if _TABLES_PATCHED:
        return
    import concourse.bacc as bacc
    from concourse import mybir

    orig = bacc.get_activation_tables
    keep = "natural_log_exp_and_others"
    shared = {
        mybir.ActivationFunctionType.Exp,
        mybir.ActivationFunctionType.Ln,
        mybir.ActivationFunctionType.Copy,
        mybir.ActivationFunctionType.Identity,
        mybir.ActivationFunctionType.Square,
    }

    def patched(arch):
        tables = orig(arch)
        for name, fns in tables.items():
            if name != keep:
                fns.difference_update(shared)
        return tables

    bacc.get_activation_tables = patched
    _TABLES_PATCHED = True


def _build(affine: bool, with_bias: bool):
    import concourse.bacc as bacc
    import concourse.tile as tile
    from concourse import mybir
    import concourse.bass as bass
    from concourse.masks import make_identity

    _patch_act_tables()
    f32 = mybir.dt.float32
    bf16 = mybir.dt.bfloat16
    nc = bacc.Bacc("TRN2", target_bir_lowering=False, debug=False, num_devices=B)

    q_d = nc.declare_dram_parameter("q", [L, D], f32, isOutput=False)
    k_d = nc.declare_dram_parameter("k", [L, D], f32, isOutput=False)
    v_d = nc.declare_dram_parameter("v", [L, D], f32, isOutput=False)
    res_d = nc.declare_dram_parameter("residual", [L, DM], f32, isOutput=False)
    fcw_d = nc.declare_dram_parameter("fc_w", [DM, D], f32, isOutput=False)
    fcb_d = nc.declare_dram_parameter("fc_b", [DM], f32, isOutput=False)
    gam_d = nc.declare_dram_parameter("ln_gamma", [DM], f32, isOutput=False)
    bet_d = nc.declare_dram_parameter("ln_beta", [DM], f32, isOutput=False)
    out_d = nc.declare_dram_parameter("out", [L, DM], f32, isOutput=True)

    with tile.TileContext(nc) as tc:
        with (
            tc.tile_pool(name="raw", bufs=2) as raw_pool,
            tc.tile_pool(name="persist", bufs=1) as persist,
            tc.tile_pool(name="stage", bufs=2, space="PSUM") as stage_pool,
            tc.tile_pool(name="pv", bufs=2, space="PSUM") as pv_pool,
            tc.tile_pool(name="fc", bufs=1, space="PSUM") as fc_pool,
            tc.tile_pool(name="et", bufs=6) as et_pool,
            tc.tile_pool(name="resid", bufs=6) as res_pool,
            tc.tile_pool(name="x", bufs=10) as x_pool,
            tc.tile_pool(name="rd", bufs=3) as rd_pool,
            tc.tile_pool(name="outs", bufs=4) as out_pool,
            tc.tile_pool(name="norm", bufs=2) as norm_pool,
            tc.tile_pool(name="small", bufs=4) as small_pool,
        ):
            identity = persist.tile([128, 128], f32)
            make_identity(nc, identity)
            one_c = persist.tile([1, 1], f32, tag="onec")
            nc.vector.memset(one_c, 1.0)

            # ---- v load first (SWDGE ring, many small descriptors) ----
            vraw = raw_pool.tile([128, NTILES, D], f32, tag="vraw")
            nc.gpsimd.dma_start(
                out=vraw, in_=v_d.ap().rearrange("(t p) d -> p t d", p=128)
            )

            # ---- tile-major loads + PE pair-transposes ----
            # qT2/kT2 [128, 16, 128] bf16: rows 0:63 = transposed data,
            # rows 64:127 = duplicate (for row-packed K=64 matmuls).
            # tile index = grp*8 + pair*2 + par
            qT2 = persist.tile([128, NTILES, 128], bf16, tag="qT")
            kT2 = persist.tile([128, NTILES, 128], bf16, tag="kT")
            for ring, (src, dstT) in zip((nc.sync, nc.scalar),
                                         ((q_d, qT2), (k_d, kT2))):
                raw = raw_pool.tile([128, NTILES, D], f32, tag="raw")
                ring.dma_start(
                    out=raw, in_=src.ap().rearrange("(t p) d -> p t d", p=128)
                )
                dlo = dstT[0:64, :, :].rearrange(
                    "d (grp pair par) c -> d grp pair par c", pair=4, par=2)
                for grp in range(NTILES // 8):
                    pt = stage_pool.tile([128, 512], f32, tag="stage")
                    for i in range(4):
                        nc.tensor.transpose(
                            pt[:, i * 128:(i + 1) * 128],
                            raw[:, (8 * grp + 2 * i): (8 * grp + 2 * i + 2), :],
                            identity,
                        )
                    ptv = pt.rearrange("p (four c) -> p four c", c=128)
                    nc.vector.tensor_copy(dlo[:, grp, :, 0, :], ptv[0:64])
                    nc.vector.tensor_copy(dlo[:, grp, :, 1, :], ptv[64:128])
                nc.gpsimd.dma_start(out=dstT[64:128, :, :],
                                    in_=dstT[0:64, :, :])

            # ---- v with ones column: [128, 16, 65] bf16 ----
            v_sb = persist.tile([128, NTILES, D + 1], bf16, tag="v")
            nc.scalar.copy(v_sb[:, :, 0:D], vraw)
            nc.vector.memset(v_sb[:, :, D:D + 1], 1.0)

            # ---- fc_wT [128, 1024] bf16, duplicated for row-packed FC ----
            fcwT = persist.tile([128, DM], bf16, tag="fcw")
            fraw = raw_pool.tile([128, DM // 128, D], f32, tag="raw")
            nc.sync.dma_start(
                out=fraw, in_=fcw_d.ap().rearrange("(t p) d -> p t d", p=128)
            )
            flo = fcwT[0:64, :].rearrange("d (pair par c) -> d pair par c",
                                          par=2, c=128)
            pt = stage_pool.tile([128, 512], f32, tag="stage")
            for i in range(4):
                nc.tensor.transpose(
                    pt[:, i * 128:(i + 1) * 128],
                    fraw[:, 2 * i: 2 * i + 2, :],
                    identity,
                )
            ptv = pt.rearrange("p (four c) -> p four c", c=128)
            nc.vector.tensor_copy(flo[:, :, 0, :], ptv[0:64])
            nc.vector.tensor_copy(flo[:, :, 1, :], ptv[64:128])
            nc.gpsimd.dma_start(out=fcwT[64:128, :], in_=fcwT[0:64, :])

            if with_bias:
                # residual gets fc_b added per tile (slow path)
                fcb_bc = persist.tile([128, DM], f32, tag="fcb")
                nc.sync.dma_start(
                    out=fcb_bc,
                    in_=bass.AP(tensor=fcb_d, offset=0, ap=[[0, 128], [1, DM]]),
                )
            if affine:
                gam_bc = persist.tile([128, DM], f32, tag="gam")
                bet_bc = persist.tile([128, DM], f32, tag="bet")
                nc.sync.dma_start(
                    out=gam_bc,
                    in_=bass.AP(tensor=gam_d, offset=0, ap=[[0, 128], [1, DM]]),
                )
                nc.sync.dma_start(
                    out=bet_bc,
                    in_=bass.AP(tensor=bet_d, offset=0, ap=[[0, 128], [1, DM]]),
                )

            state = {}

            def attention(s):
                qlo = qT2[0:64, :, :].rearrange("d t c -> d (t c)")[
                    :, s * 512:(s + 1) * 512]
                qhi = qT2[64:128, :, :].rearrange("d t c -> d (t c)")[
                    :, s * 512:(s + 1) * 512]
                out_aug = pv_pool.tile([65, 512], f32, tag="pv")
                ngrp = NTILES // 2

                def s_pair(g):
                    # row-packed: k-tile 2g in rows 0:63, 2g+1 in 64:127
                    st = stage_pool.tile([128, 1024], f32, tag="stage")
                    nc.tensor.matmul(st[:, 0:512], kT2[0:64, 2 * g, :], qlo,
                                     start=True, stop=True,
                                     tile_position=(0, 0))
                    nc.tensor.matmul(st[:, 512:1024],
                                     kT2[64:128, 2 * g + 1, :],
                                     qhi, start=True, stop=True,
                                     tile_position=(64, 0))
                    return st

                def exp_pv(g, st):
                    et = et_pool.tile([128, 1024], bf16, tag="et")
                    nc.scalar.activation(
                        out=et, in_=st,
                        func=mybir.ActivationFunctionType.Exp, scale=SCALE,
                    )
                    nc.tensor.matmul(out_aug, v_sb[:, 2 * g, :], et[:, 0:512],
                                     start=(g == 0), stop=False)
                    nc.tensor.matmul(out_aug, v_sb[:, 2 * g + 1, :],
                                     et[:, 512:1024],
                                     start=False, stop=(g == ngrp - 1))

                # S one group ahead so the PE never waits on exp
                st_prev = s_pair(0)
                for g in range(1, ngrp):
                    st_cur = s_pair(g)
                    exp_pv(g - 1, st_prev)
                    st_prev = st_cur
                exp_pv(ngrp - 1, st_prev)
                return out_aug

            def dance(s, out_aug):
                # f32 denominator row, then evacuate + duplicate (bf16)
                drow = small_pool.tile([1, 512], f32, tag="drow")
                nc.vector.tensor_copy(drow, out_aug[64:65, :])
                outU = norm_pool.tile([128, 512], bf16, tag="outU")
                nc.vector.tensor_copy(outU[0:64, :], out_aug[0:64, :])
                nc.scalar.dma_start(out=outU[64:128, :], in_=outU[0:64, :])
                # D -> per-partition [128, 4] via 4 tiny K=1 PE matmuls
                dps = stage_pool.tile([128, 4], f32, tag="stage")
                for t in range(4):
                    nc.tensor.matmul(dps[:, t:t + 1],
                                     drow[:, t * 128:(t + 1) * 128], one_c,
                                     start=True, stop=True)
                dT = small_pool.tile([128, 4], f32, tag="dT")
                nc.vector.tensor_copy(dT, dps)
                # eps' = D^2 * eps (per-partition epsilon for LN on y=D*x)
                epsT = small_pool.tile([128, 4], f32, tag="epsT")
                nc.vector.tensor_mul(epsT, dT, dT)
                nc.vector.tensor_scalar_mul(out=epsT, in0=epsT,
                                            scalar1=LN_EPS)
                state[s] = {"outU": outU, "dT": dT, "epsT": epsT}

            def epilogue_a(s):
                outU = state[s]["outU"]
                dT = state[s]["dT"]
                mv_all = small_pool.tile([128, 4, 2], f32, tag="mv")
                x_ts = []
                for pi in range(4):
                    t = s * 4 + pi
                    fc_ps = fc_pool.tile([128, DM], f32, tag="fc")
                    nc.tensor.matmul(fc_ps[:, 0:512],
                                     outU[0:64, pi * 128:(pi + 1) * 128],
                                     fcwT[0:64, 0:512],
                                     start=True, stop=True,
                                     tile_position=(0, 0))
                    nc.tensor.matmul(fc_ps[:, 512:1024],
                                     outU[64:128, pi * 128:(pi + 1) * 128],
                                     fcwT[64:128, 512:1024],
                                     start=True, stop=True,
                                     tile_position=(64, 0))
                    res_t = res_pool.tile([128, DM], f32, tag="res")
                    nc.sync.dma_start(
                        out=res_t, in_=res_d[t * 128:(t + 1) * 128, :]
                    )
                    if with_bias:
                        nc.vector.tensor_add(res_t, res_t, fcb_bc)
                    # y = u + D*(b + r); LN(y) == LN(u/D + b + r)
                    rD = rd_pool.tile([128, DM], f32, tag="rd")
                    nc.vector.tensor_scalar_mul(out=rD, in0=res_t,
                                                scalar1=dT[:, pi:pi + 1])
                    x_t = x_pool.tile([128, DM], f32, tag="x")
                    nc.vector.tensor_add(x_t, fc_ps, rD)
                    x_ts.append(x_t)
                    stats = small_pool.tile([128, 2, 6], f32, tag="stats")
                    nc.vector.bn_stats(out=stats[:, 0, :], in_=x_t[:, 0:512])
                    nc.vector.bn_stats(out=stats[:, 1, :],
                                       in_=x_t[:, 512:1024])
                    nc.vector.bn_aggr(out=mv_all[:, pi, :], in_=stats)
                state[s]["mv"] = mv_all
                state[s]["x_ts"] = x_ts

            def epilogue_b(s):
                epsT = state[s]["epsT"]
                mv_all = state[s]["mv"]
                x_ts = state[s]["x_ts"]
                # batched rsqrt: rstd = exp(-0.5*ln(var + D^2 eps))
                var4 = small_pool.tile([128, 4], f32, tag="var4")
                nc.vector.tensor_add(var4, mv_all[:, :, 1], epsT)
                rstd4 = small_pool.tile([128, 4], f32, tag="rstd")
                nc.scalar.activation(
                    out=rstd4, in_=var4,
                    func=mybir.ActivationFunctionType.Ln,
                )
                nc.scalar.activation(
                    out=rstd4, in_=rstd4,
                    func=mybir.ActivationFunctionType.Exp, scale=-0.5,
                )
                nm4 = small_pool.tile([128, 4], f32, tag="nm")
                nc.vector.tensor_tensor(
                    out=nm4, in0=mv_all[:, :, 0], in1=rstd4,
                    op=mybir.AluOpType.mult,
                )
                nc.vector.tensor_scalar_mul(out=nm4, in0=nm4, scalar1=-1.0)

                for pi in range(4):
                    t = s * 4 + pi
                    out_t = out_pool.tile([128, DM], f32, tag="out")
                    if pi % 2 == 0:
                        nc.vector.tensor_scalar(
                            out=out_t, in0=x_ts[pi],
                            scalar1=mv_all[:, pi, 0:1],
                            scalar2=rstd4[:, pi:pi + 1],
                            op0=mybir.AluOpType.subtract,
                            op1=mybir.AluOpType.mult,
                        )
                    else:
                        nc.scalar.activation(
                            out=out_t, in_=x_ts[pi],
                            func=mybir.ActivationFunctionType.Identity,
                            bias=nm4[:, pi:pi + 1],
                            scale=rstd4[:, pi:pi + 1],
                        )
                    if affine:
                        nc.vector.tensor_mul(out_t, out_t, gam_bc)
                        nc.vector.tensor_add(out_t, out_t, bet_bc)
                    nc.gpsimd.dma_start(
                        out=out_d[t * 128:(t + 1) * 128, :], in_=out_t
                    )
                del state[s]

            # pipeline: previous slice's dance + epilogue_a are emitted
            # before the next attention (front of the FIFOs); the LN applies
            # (epilogue_b) go one further iteration later so they never
            # block the following slice's exps on the ScalarE FIFO
            oa = {}
            for s in range(NSLICES + 2):
                if s - 2 >= 0:
                    epilogue_b(s - 2)
                if s - 1 >= 0 and (s - 1) < NSLICES:
                    dance(s - 1, oa.pop(s - 1))
                    epilogue_a(s - 1)
                if s < NSLICES:
                    oa[s] = attention(s)

    nc.finalize()
    return nc


LAST_RESULTS = None


def kernel(q, k, v, residual, fc_w, fc_b, ln_gamma, ln_beta):
    from concourse.bass_utils import run_bass_kernel_spmd

    global LAST_RESULTS
    affine = not (
        np.allclose(ln_gamma, 1.0) and np.allclose(ln_beta, 0.0)
    )
    with_bias = not np.all(np.asarray(fc_b) == 0.0)
    key = ("v10", affine, with_bias)
    if key not in _CACHE:
        _CACHE[key] = _build(affine, with_bias)
    nc = _CACHE[key]

    q = np.ascontiguousarray(q, dtype=np.float32)
    k = np.ascontiguousarray(k, dtype=np.float32)
    v = np.ascontiguousarray(v, dtype=np.float32)
    residual = np.ascontiguousarray(residual, dtype=np.float32)
    fc_w = np.ascontiguousarray(fc_w, dtype=np.float32)
    fc_b = np.ascontiguousarray(fc_b, dtype=np.float32)
    ln_gamma = np.ascontiguousarray(ln_gamma, dtype=np.float32)
    ln_beta = np.ascontiguousarray(ln_beta, dtype=np.float32)

    in_maps = [
        {
            "q": q[b], "k": k[b], "v": v[b], "residual": residual[b],
            "fc_w": fc_w, "fc_b": fc_b,
            "ln_gamma": ln_gamma, "ln_beta": ln_beta,
        }
        for b in range(B)
    ]
    res = run_bass_kernel_spmd(nc, in_maps, core_ids=list(range(B)))
    LAST_RESULTS = res
    return np.stack([res.results[b]["out"] for b in range(B)], axis=0)


# revision 26
# speedup vs baseline: 1.3572x; 1.0151x over previous
"""Fused attention + FC + residual + LayerNorm for Trainium2, 8 NeuronCores.

Problem: B=8, L=2048, d_k=d_v=64, d_model=1024, fp32 I/O.
Sharding: pure data parallel - batch element b -> core b. No collectives.

Key algebraic trick: LayerNorm is scale-invariant, so the softmax
normalization is never applied. With u = PV_unnorm @ W^T (unnormalized
attention output through the FC) and D the per-row softmax denominator:

    LN(u/D + b + r)  ==  LN(u + D*(b + r))

so instead of dividing by D (which needs a reciprocal plus a costly
cross-partition broadcast), the kernel scales the residual by D - a cheap
per-partition tensor_scalar - and lets LN absorb the factor. Only the LN
epsilon needs rescaling (eps' = D^2 * eps), also per-partition.

This target's PE streams at 1.2 GHz (HAM never unthrottles), so matmul cost
is ~0.82 ns/column + ~73 ns/instruction; all K=64 matmuls are row-packed
(two concurrent matmuls in row groups 0-63 / 64-127 via tile_position) at
the cost of duplicating qT/kT/outU/fc_wT across both partition halves.

Software pipeline (engine queues are strict FIFO):

  iter s:  dance(s-1) -> epilogue(s-1) -> attention(s)

  attention(s): per k-tile pair, S^T [128k, 2x512q] via one row-packed
    matmul pair (bf16, f32 PSUM) -> exp on ScalarE (temperature 1/sqrt(64)
    folded into the free affine scale, bf16 out) -> PV matmul accumulates
    [65, 512] f32, row 64 = softmax denominator (ones-column appended to V).
  dance(s): D row -> SBUF f32; out_aug evacuated to bf16 outU (frees the
    PSUM bank immediately) + duplicated across partition halves; D
    transposed to per-partition [128, 4] via 4 tiny K=1 PE matmuls (no
    DRAM roundtrip); eps' = D^2*eps.
  epilogue(s): row-packed FC -> residual scaled by D (DVE tensor_scalar)
    -> added to FC PSUM -> bn_stats/bn_aggr -> rsqrt batched per slice as
    Ln/Exp on ScalarE (single ACT table set for the whole kernel) -> LN
    apply alternating DVE tensor_scalar / ScalarE Identity -> store on the
    GPSIMD SWDGE ring.
"""
import numpy as np

B = 8
L = 2048
D = 64
DM = 1024
NTILES = L // 128       # 16 q/k tiles of 128
NSLICES = L // 512      # 4 q-slices of 512
LN_EPS = 1e-5
SCALE = 0.125           # 1/sqrt(64)

_CACHE = {}
_TABLES_PATCHED = False


def _patch_act_tables():
    """Force every activation we use into one table set so the scheduler
    never needs a mid-kernel ACT_TABLE_LOAD switch (Exp <-> Ln)."""
    global _TABLES_PATCHED
    if _TABLES_PATCHED:
        return
    import concourse.bacc as bacc
    from concourse import mybir

    orig = bacc.get_activation_tables
    keep = "natural_log_exp_and_others"
    shared = {
        mybir.ActivationFunctionType.Exp,
        mybir.ActivationFunctionType.Ln,
        mybir.ActivationFunctionType.Copy,
        mybir.ActivationFunctionType.Identity,
        mybir.ActivationFunctionType.Square,
    }

    def patched(arch):
        tables = orig(arch)
        for name, fns in tables.items():
            if name != keep:
                fns.difference_update(shared)
        return tables

    bacc.get_activation_tables = patched
    _TABLES_PATCHED = True


def _build(affine: bool, with_bias: bool):
    import concourse.bacc as bacc
    import concourse.tile as tile
    from concourse import mybir
    import concourse.bass as bass
    from concourse.masks import make_identity

    _patch_act_tables()
    f32 = mybir.dt.float32
    bf16 = mybir.dt.bfloat16
    nc = bacc.Bacc("TRN2", target_bir_lowering=False, debug=False, num_devices=B)

    q_d = nc.declare_dram_parameter("q", [L, D], f32, isOutput=False)
    k_d = nc.declare_dram_parameter("k", [L, D], f32, isOutput=False)
    v_d = nc.declare_dram_parameter("v", [L, D], f32, isOutput=False)
    res_d = nc.declare_dram_parameter("residual", [L, DM], f32, isOutput=False)
    fcw_d = nc.declare_dram_parameter("fc_w", [DM, D], f32, isOutput=False)
    fcb_d = nc.declare_dram_parameter("fc_b", [DM], f32, isOutput=False)
    gam_d = nc.declare_dram_parameter("ln_gamma", [DM], f32, isOutput=False)
    bet_d = nc.declare_dram_parameter("ln_beta", [DM], f32, isOutput=False)
    out_d = nc.declare_dram_parameter("out", [L, DM], f32, isOutput=True)

    with tile.TileContext(nc) as tc:
        with (
            tc.tile_pool(name="raw", bufs=2) as raw_pool,
            tc.tile_pool(name="persist", bufs=1) as persist,
            tc.tile_pool(name="stage", bufs=2, space="PSUM") as stage_pool,
            tc.tile_pool(name="pv", bufs=2, space="PSUM") as pv_pool,
            tc.tile_pool(name="fc", bufs=1, space="PSUM") as fc_pool,
            tc.tile_pool(name="et", bufs=6) as et_pool,
            tc.tile_pool(name="resid", bufs=6) as res_pool,
            tc.tile_pool(name="x", bufs=10) as x_pool,
            tc.tile_pool(name="rd", bufs=3) as rd_pool,
            tc.tile_pool(name="outs", bufs=4) as out_pool,
            tc.tile_pool(name="norm", bufs=2) as norm_pool,
            tc.tile_pool(name="small", bufs=4) as small_pool,
        ):
            identity = persist.tile([128, 128], f32)
            make_identity(nc, identity)
            one_c = persist.tile([1, 1], f32, tag="onec")
            nc.vector.memset(one_c, 1.0)

            # ---- v load first (SWDGE ring, many small descriptors) ----
            vraw = raw_pool.tile([128, NTILES, D], f32, tag="vraw")
            nc.gpsimd.dma_start(
                out=vraw, in_=v_d.ap().rearrange("(t p) d -> p t d", p=128)
            )

            # ---- tile-major loads + PE pair-transposes ----
            # qT2/kT2 [128, 16, 128] bf16: rows 0:63 = transposed data,
            # rows 64:127 = duplicate (for row-packed K=64 matmuls).
            # tile index = grp*8 + pair*2 + par
            qT2 = persist.tile([128, NTILES, 128], bf16, tag="qT")
            kT2 = persist.tile([128, NTILES, 128], bf16, tag="kT")
            for ring, (src, dstT) in zip((nc.sync, nc.scalar),
                                         ((q_d, qT2), (k_d, kT2))):
                raw = raw_pool.tile([128, NTILES, D], f32, tag="raw")
                ring.dma_start(
                    out=raw, in_=src.ap().rearrange("(t p) d -> p t d", p=128)
                )
                dlo = dstT[0:64, :, :].rearrange(
                    "d (grp pair par) c -> d grp pair par c", pair=4, par=2)
                for grp in range(NTILES // 8):
                    pt = stage_pool.tile([128, 512], f32, tag="stage")
                    for i in range(4):
                        nc.tensor.transpose(
                            pt[:, i * 128:(i + 1) * 128],
                            raw[:, (8 * grp + 2 * i): (8 * grp + 2 * i + 2), :],
                            identity,
                        )
                    ptv = pt.rearrange("p (four c) -> p four c", c=128)
                    nc.vector.tensor_copy(dlo[:, grp, :, 0, :], ptv[0:64])
                    nc.vector.tensor_copy(dlo[:, grp, :, 1, :], ptv[64:128])
                    ring.dma_start(
                        out=dstT[64:128, 8 * grp:8 * grp + 8, :],
                        in_=dstT[0:64, 8 * grp:8 * grp + 8, :],
                    )

            # ---- v with ones column: [128, 16, 65] bf16 ----
            v_sb = persist.tile([128, NTILES, D + 1], bf16, tag="v")
            nc.scalar.copy(v_sb[:, :, 0:D], vraw)
            nc.vector.memset(v_sb[:, :, D:D + 1], 1.0)

            # ---- fc_wT [128, 1024] bf16, duplicated for row-packed FC ----
            fcwT = persist.tile([128, DM], bf16, tag="fcw")
            fraw = raw_pool.tile([128, DM // 128, D], f32, tag="raw")
            nc.sync.dma_start(
                out=fraw, in_=fcw_d.ap().rearrange("(t p) d -> p t d", p=128)
            )
            flo = fcwT[0:64, :].rearrange("d (pair par c) -> d pair par c",
                                          par=2, c=128)
            pt = stage_pool.tile([128, 512], f32, tag="stage")
            for i in range(4):
                nc.tensor.transpose(
                    pt[:, i * 128:(i + 1) * 128],
                    fraw[:, 2 * i: 2 * i + 2, :],
                    identity,
                )
            ptv = pt.rearrange("p (four c) -> p four c", c=128)
            nc.vector.tensor_copy(flo[:, :, 0, :], ptv[0:64])
            nc.vector.tensor_copy(flo[:, :, 1, :], ptv[64:128])
            nc.gpsimd.dma_start(out=fcwT[64:128, :], in_=fcwT[0:64, :])

            if with_bias:
                # residual gets fc_b added per tile (slow path)
                fcb_bc = persist.tile([128, DM], f32, tag="fcb")
                nc.sync.dma_start(
                    out=fcb_bc,
                    in_=bass.AP(tensor=fcb_d, offset=0, ap=[[0, 128], [1, DM]]),
                )
            if affine:
                gam_bc = persist.tile([128, DM], f32, tag="gam")
                bet_bc = persist.tile([128, DM], f32, tag="bet")
                nc.sync.dma_start(
                    out=gam_bc,
                    in_=bass.AP(tensor=gam_d, offset=0, ap=[[0, 128], [1, DM]]),
                )
                nc.sync.dma_start(
                    out=bet_bc,
                    in_=bass.AP(tensor=bet_d, offset=0, ap=[[0, 128], [1, DM]]),
                )

            state = {}

            def attention(s):
                qlo = qT2[0:64, :, :].rearrange("d t c -> d (t c)")[
                    :, s * 512:(s + 1) * 512]
                qhi = qT2[64:128, :, :].rearrange("d t c -> d (t c)")[
                    :, s * 512:(s + 1) * 512]
                out_aug = pv_pool.tile([65, 512], f32, tag="pv")
                ngrp = NTILES // 2

                def s_pair(g):
                    # row-packed: k-tile 2g in rows 0:63, 2g+1 in 64:127
                    st = stage_pool.tile([128, 1024], f32, tag="stage")
                    nc.tensor.matmul(st[:, 0:512], kT2[0:64, 2 * g, :], qlo,
                                     start=True, stop=True,
                                     tile_position=(0, 0))
                    nc.tensor.matmul(st[:, 512:1024],
                                     kT2[64:128, 2 * g + 1, :],
                                     qhi, start=True, stop=True,
                                     tile_position=(64, 0))
                    return st

                def exp_pv(g, st):
                    et = et_pool.tile([128, 1024], bf16, tag="et")
                    nc.scalar.activation(
                        out=et, in_=st,
                        func=mybir.ActivationFunctionType.Exp, scale=SCALE,
                    )
                    nc.tensor.matmul(out_aug, v_sb[:, 2 * g, :], et[:, 0:512],
                                     start=(g == 0), stop=False)
                    nc.tensor.matmul(out_aug, v_sb[:, 2 * g + 1, :],
                                     et[:, 512:1024],
                                     start=False, stop=(g == ngrp - 1))

                # S one group ahead so the PE never waits on exp
                st_prev = s_pair(0)
                for g in range(1, ngrp):
                    st_cur = s_pair(g)
                    exp_pv(g - 1, st_prev)
                    st_prev = st_cur
                exp_pv(ngrp - 1, st_prev)
                return out_aug

            def dance(s, out_aug):
                # f32 denominator row, then evacuate + duplicate (bf16)
                drow = small_pool.tile([1, 512], f32, tag="drow")
                nc.vector.tensor_copy(drow, out_aug[64:65, :])
                outU = norm_pool.tile([128, 512], bf16, tag="outU")
                nc.vector.tensor_copy(outU[0:64, :], out_aug[0:64, :])
                nc.scalar.dma_start(out=outU[64:128, :], in_=outU[0:64, :])
                # D -> per-partition [128, 4] via 4 tiny K=1 PE matmuls
                dps = stage_pool.tile([128, 4], f32, tag="stage")
                for t in range(4):
                    nc.tensor.matmul(dps[:, t:t + 1],
                                     drow[:, t * 128:(t + 1) * 128], one_c,
                                     start=True, stop=True)
                dT = small_pool.tile([128, 4], f32, tag="dT")
                nc.vector.tensor_copy(dT, dps)
                # eps' = D^2 * eps (per-partition epsilon for LN on y=D*x)
                epsT = small_pool.tile([128, 4], f32, tag="epsT")
                nc.vector.tensor_mul(epsT, dT, dT)
                nc.vector.tensor_scalar_mul(out=epsT, in0=epsT,
                                            scalar1=LN_EPS)
                state[s] = {"outU": outU, "dT": dT, "epsT": epsT}

            def epilogue_a(s):
                outU = state[s]["outU"]
                dT = state[s]["dT"]
                mv_all = small_pool.tile([128, 4, 2], f32, tag="mv")
                x_ts = []
                for pi in range(4):
                    t = s * 4 + pi
                    fc_ps = fc_pool.tile([128, DM], f32, tag="fc")
                    nc.tensor.matmul(fc_ps[:, 0:512],
                                     outU[0:64, pi * 128:(pi + 1) * 128],
                                     fcwT[0:64, 0:512],
                                     start=True, stop=True,
                                     tile_position=(0, 0))
                    nc.tensor.matmul(fc_ps[:, 512:1024],
                                     outU[64:128, pi * 128:(pi + 1) * 128],
                                     fcwT[64:128, 512:1024],
                                     start=True, stop=True,
                                     tile_position=(64, 0))
                    res_t = res_pool.tile([128, DM], f32, tag="res")
                    nc.sync.dma_start(
                        out=res_t, in_=res_d[t * 128:(t + 1) * 128, :]
                    )
                    if with_bias:
                        nc.vector.tensor_add(res_t, res_t, fcb_bc)
                    # y = u + D*(b + r); LN(y) == LN(u/D + b + r)
                    rD = rd_pool.tile([128, DM], f32, tag="rd")
                    nc.vector.tensor_scalar_mul(out=rD, in0=res_t,
                                                scalar1=dT[:, pi:pi + 1])
                    x_t = x_pool.tile([128, DM], f32, tag="x")
                    nc.vector.tensor_add(x_t, fc_ps, rD)
                    x_ts.append(x_t)
                    stats = small_pool.tile([128, 2, 6], f32, tag="stats")
                    nc.vector.bn_stats(out=stats[:, 0, :], in_=x_t[:, 0:512])
                    nc.vector.bn_stats(out=stats[:, 1, :],
                                       in_=x_t[:, 512:1024])
                    nc.vector.bn_aggr(out=mv_all[:, pi, :], in_=stats)
                state[s]["mv"] = mv_all
                state[s]["x_ts"] = x_ts

            def epilogue_b(s):
                epsT = state[s]["epsT"]
                mv_all = state[s]["mv"]
                x_ts = state[s]["x_ts"]
                # batched rsqrt: rstd = exp(-0.5*ln(var + D^2 eps))
                var4 = small_pool.tile([128, 4], f32, tag="var4")
                nc.vector.tensor_add(var4, mv_all[:, :, 1], epsT)
                rstd4 = small_pool.tile([128, 4], f32, tag="rstd")
                nc.scalar.activation(
                    out=rstd4, in_=var4,
                    func=mybir.ActivationFunctionType.Ln,
                )
                nc.scalar.activation(
                    out=rstd4, in_=rstd4,
                    func=mybir.ActivationFunctionType.Exp, scale=-0.5,
                )
                nm4 = small_pool.tile([128, 4], f32, tag="nm")
                nc.vector.tensor_tensor(
                    out=nm4, in0=mv_all[:, :, 0], in1=rstd4,
                    op=mybir.AluOpType.mult,
                )
                nc.vector.tensor_scalar_mul(out=nm4, in0=nm4, scalar1=-1.0)

                for pi in range(4):
                    t = s * 4 + pi
                    out_t = out_pool.tile([128, DM], f32, tag="out")
                    if pi % 2 == 0:
                        nc.vector.tensor_scalar(
                            out=out_t, in0=x_ts[pi],
                            scalar1=mv_all[:, pi, 0:1],
                            scalar2=rstd4[:, pi:pi + 1],
                            op0=mybir.AluOpType.subtract,
                            op1=mybir.AluOpType.mult,
                        )
                    else:
                        nc.scalar.activation(
                            out=out_t, in_=x_ts[pi],
                            func=mybir.ActivationFunctionType.Identity,
                            bias=nm4[:, pi:pi + 1],
                            scale=rstd4[:, pi:pi + 1],
                        )
                    if affine:
                        nc.vector.tensor_mul(out_t, out_t, gam_bc)
                        nc.vector.tensor_add(out_t, out_t, bet_bc)
                    nc.gpsimd.dma_start(
                        out=out_d[t * 128:(t + 1) * 128, :], in_=out_t
                    )
                del state[s]

            # pipeline: previous slice's dance + epilogue_a are emitted
            # before the next attention (front of the FIFOs); the LN applies
            # (epilogue_b) go one further iteration later so they never
            # block the following slice's exps on the ScalarE FIFO
            oa = {}
            for s in range(NSLICES + 1):
                if s - 1 >= 0 and (s - 1) < NSLICES:
                    dance(s - 1, oa.pop(s - 1))
                    epilogue_a(s - 1)
                if s < NSLICES:
                    oa[s] = attention(s)
                if s - 1 >= 0:
                    epilogue_b(s - 1)

    nc.finalize()
    return nc


LAST_RESULTS = None


def kernel(q, k, v, residual, fc_w, fc_b, ln_gamma, ln_beta):
    from concourse.bass_utils import run_bass_kernel_spmd

    global LAST_RESULTS
    affine = not (
        np.allclose(ln_gamma, 1.0) and np.allclose(ln_beta, 0.0)
    )
    with_bias = not np.all(np.asarray(fc_b) == 0.0)
    key = ("v12", affine, with_bias)
    if key not in _CACHE:
        _CACHE[key] = _build(affine, with_bias)
    nc = _CACHE[key]

    q = np.ascontiguousarray(q, dtype=np.float32)
    k = np.ascontiguousarray(k, dtype=np.float32)
    v = np.ascontiguousarray(v, dtype=np.float32)
    residual = np.ascontiguousarray(residual, dtype=np.float32)
    fc_w = np.ascontiguousarray(fc_w, dtype=np.float32)
    fc_b = np.ascontiguousarray(fc_b, dtype=np.float32)
    ln_gamma = np.ascontiguousarray(ln_gamma, dtype=np.float32)
    ln_beta = np.ascontiguousarray(ln_beta, dtype=np.float32)

    in_maps = [
        {
            "q": q[b], "k": k[b], "v": v[b], "residual": residual[b],
            "fc_w": fc_w, "fc_b": fc_b,
            "ln_gamma": ln_gamma, "ln_beta": ln_beta,
        }
        for b in range(B)
    ]
    res = run_bass_kernel_spmd(nc, in_maps, core_ids=list(range(B)))
    LAST_RESULTS = res
    return np.stack([res.results[b]["out"] for b in range(B)], axis=0)
